# revision 51
# baseline (speedup 1.0000x reference)
"""Trainium2 Bass kernel for 16-head causal MHA (B=4, S=2048, D=1024).

Sharding: 8 cores = 4 batches x 2 head-groups (8 heads each).
Per core (batch b, head-group hg):
  inputs:  XT = X[b].T [1024,2048] bf16, WQ/WK/WV column shards [1024,512]
           bf16, WO row shard [512,1024] bf16, bias shards, causal mask tile.
  output:  YT = (O_hg @ WO_hg + bo*[hg==0]).T  [1024, 2048]  (partial)
Host combine: Y[b] = (YT[2b] + YT[2b+1]).T

On-core dataflow (all-bf16 matmul pipeline, fp32 PSUM):
  Q^T,K^T [512,2048] and V [2048,512] via bf16 matmuls.
  scores^T[sk,sq] = K_h @ Q_h^T (bf16, exact causal diagonal trim); exp on
  ACT (scale=1/8 folded) -> A^T in bf16; causal mask multiplied on the
  diagonal 128-block (DVE).
  AV in the M=128 form: O[sq,dv+1] += A^T-chunk(stationary) @ [V|1](moving)
  in bf16 (N=65); col 64 gives the softmax denominators for free.
  Normalize on DVE (per-partition reciprocal scale), O -> O^T via PE
  transposes (identity; 4 blocks share one lazily-zeroed PSUM bank),
  Y^T = WO^T @ O^T in bf16.
  Projection / output-projection matmuls are interleaved into the attention
  chunk stream as fillers so the PE never stalls on the ACT exp chain.
"""

import sys
from collections import deque

import numpy as np

_REPO = "/opt/trn_rl_repo"

B, S, D = 4, 2048, 1024
H, DK = 16, 64
HPC = 8            # heads per core
MD = HPC * DK      # 512: per-core head width
P = 128
SB = 512           # s-block
NSB = S // SB      # 4
NDC = D // P       # 8
NMC = MD // P      # 4
NSC = S // P       # 16

_CACHE = {}


def _ensure_path():
    try:
        import concourse  # noqa: F401
    except ImportError:
        if _REPO not in sys.path:
            sys.path.insert(0, _REPO)


def _build():
    _ensure_path()
    from contextlib import ExitStack

    import concourse.bass as bass  # noqa: F401
    import concourse.mybir as mybir
    import concourse.tile as tile
    from concourse import bacc

    dt = mybir.dt
    f32 = dt.float32
    bf16 = dt.bfloat16
    AF = mybir.ActivationFunctionType

    nc = bacc.Bacc(None, target_bir_lowering=False)
    # All inputs host-pre-laid into per-partition-contiguous layouts so every
    # load is one fat DMA (descriptor rows >= 512B avoid the 2x RMW penalty;
    # each DMA costs ~650ns sequencer + ~900ns semaphore regardless of size).
    XT = nc.dram_tensor("XT", [P, NDC, S], bf16, kind="ExternalInput")
    WQ = nc.dram_tensor("WQ", [P, NMC, NDC, P], bf16, kind="ExternalInput")
    WK = nc.dram_tensor("WK", [P, NMC, NDC, P], bf16, kind="ExternalInput")
    WV = nc.dram_tensor("WV", [P, NDC, MD], bf16, kind="ExternalInput")
    WO = nc.dram_tensor("WO", [P, NMC, D], bf16, kind="ExternalInput")
    # packed consts: CF32 = [bq | bk | bo] per-partition cols; CBF =
    # [masks | ident | vb-broadcast]
    CF32 = nc.dram_tensor("CF32", [P, 2 * NMC + NDC], f32, kind="ExternalInput")
    CBF = nc.dram_tensor("CBF", [P, 2 * P + MD], bf16, kind="ExternalInput")
    YT = nc.dram_tensor("YT", [D, S], bf16, kind="ExternalOutput")

    with ExitStack() as ctx:
        ctx.enter_context(nc.allow_low_precision(reason="bf16 pipeline"))
        tc = ctx.enter_context(tile.TileContext(nc))
        consts = ctx.enter_context(tc.tile_pool(name="consts", bufs=1))
        qkv = ctx.enter_context(tc.tile_pool(name="qkv", bufs=1))
        wst = ctx.enter_context(tc.tile_pool(name="wst", bufs=1))
        xtp = ctx.enter_context(tc.tile_pool(name="xt", bufs=2))
        qtp = ctx.enter_context(tc.tile_pool(name="qt", bufs=2))
        attnp = ctx.enter_context(tc.tile_pool(name="attn", bufs=3))
        osbp = ctx.enter_context(tc.tile_pool(name="osb", bufs=2))
        otp = ctx.enter_context(tc.tile_pool(name="ot", bufs=4))
        recp = ctx.enter_context(tc.tile_pool(name="rec", bufs=4))
        ybp = ctx.enter_context(tc.tile_pool(name="yb", bufs=8))
        pps = ctx.enter_context(tc.tile_pool(name="pps", bufs=2, space="PSUM"))
        psp = ctx.enter_context(tc.tile_pool(name="psp", bufs=2, space="PSUM"))
        pav = ctx.enter_context(tc.tile_pool(name="pav", bufs=2, space="PSUM"))

        # Dummy first ACT op: walrus attaches the ACT table-load pseudo to the
        # first activation; keep its sync-wait list minimal.
        dummy = consts.tile([1, 16], f32)
        nc.vector.memset(dummy[:], 0.0)
        nc.scalar.activation(dummy[:], dummy[:], AF.Exp)
        nc.scalar.activation(dummy[:], dummy[:], AF.Identity)

        kt = qkv.tile([P, NMC, S], bf16)              # K^T  (m-chunk, sk)
        vaug = qkv.tile([P, NSC, HPC, DK + 1], bf16)  # V per s-chunk + ones col
        wo_bf = qkv.tile([P, NMC, D], bf16)

        # ---------- setup DMAs, ordered so the PE starts early --------------
        # Two HWDGE queues: ACT (idle until the first exp at ~10us) carries
        # the first W strips + consts; SP carries the xt stream and all
        # later bulk loads so the exp stream never queues behind a DMA.
        xt_t = [None] * NSB
        xt_t[0] = xtp.tile([P, NDC, SB], bf16, name="xtblk")
        wq_t = wst.tile([P, NMC, NDC, P], bf16, tag="wq")
        wk_t = wst.tile([P, NMC, NDC, P], bf16, tag="wk")
        wv_t = wst.tile([P, NDC, MD], bf16, tag="wv")

        nc.sync.dma_start(xt_t[0][:, 0, :], XT[:, 0, 0:SB])
        nc.scalar.dma_start(wq_t[:, 0, 0:4, :], WQ[:, 0, 0:4, :])
        nc.sync.dma_start(xt_t[0][:, 1, :], XT[:, 1, 0:SB])
        nc.scalar.dma_start(wq_t[:, 0, 4:NDC, :], WQ[:, 0, 4:NDC, :])
        nc.sync.dma_start(xt_t[0][:, 2:4, :], XT[:, 2:4, 0:SB])
        nc.scalar.dma_start(wk_t[:, 0, :, :], WK[:, 0, :, :])
        nc.sync.dma_start(xt_t[0][:, 4:NDC, :], XT[:, 4:NDC, 0:SB])
        cf = consts.tile([P, 2 * NMC + NDC], f32)
        nc.scalar.dma_start(cf[:], CF32[:, :])
        bqt = cf[:, 0:NMC]
        bkt = cf[:, NMC:2 * NMC]
        bot = cf[:, 2 * NMC:]
        nc.scalar.dma_start(wq_t[:, 1, :, :], WQ[:, 1, :, :])
        nc.scalar.dma_start(wk_t[:, 1, :, :], WK[:, 1, :, :])
        cb = consts.tile([P, 2 * P + MD], bf16)
        nc.scalar.dma_start(cb[:], CBF[:, :])
        masks_b = cb[:, 0:P]
        ident_b = cb[:, P:2 * P]
        vb_sb = cb[:, 2 * P:].rearrange("p (h d) -> p h d", h=HPC)
        # remaining bulk loads on SP, in PE-consumption order
        nc.sync.dma_start(wq_t[:, 2, :, :], WQ[:, 2, :, :])
        nc.sync.dma_start(wk_t[:, 2, :, :], WK[:, 2, :, :])
        nc.sync.dma_start(wq_t[:, 3, :, :], WQ[:, 3, :, :])
        nc.sync.dma_start(wk_t[:, 3, :, :], WK[:, 3, :, :])
        nc.sync.dma_start(wv_t[:, 0:4, :], WV[:, 0:4, :])
        xt_t[1] = xtp.tile([P, NDC, SB], bf16, name="xtblk")
        nc.sync.dma_start(xt_t[1][:, 0:4, :], XT[:, 0:4, SB:2 * SB])
        nc.sync.dma_start(wv_t[:, 4:NDC, :], WV[:, 4:NDC, :])
        nc.sync.dma_start(xt_t[1][:, 4:NDC, :], XT[:, 4:NDC, SB:2 * SB])
        nc.sync.dma_start(wo_bf[:], WO[:, :, :])

        # ---------- emitters ------------------------------------------------
        # filler units: (estimated_pe_ns, deadline, closure)
        U = 0.41667  # ns per PE row at full clock

        def qk_units(sb, qt):
            """Q/K projection for s-block sb: 16 (est, closure) units."""
            xt = xt_t[sb]

            def qk_half(w_t, bias_t, out_t, mc, half, ps_box):
                def run():
                    if half == 0:
                        ps_box[0] = pps.tile([P, SB], f32, name="ps")
                    ps = ps_box[0]
                    for dc in range(4 * half, 4 * half + 4):
                        nc.tensor.matmul(
                            ps[:],
                            (w_t[:, mc, dc, :]),
                            (xt[:, dc, :]),
                            start=(dc == 0),
                            stop=(dc == NDC - 1),
                        )
                    if half == 1:
                        nc.vector.tensor_scalar_add(
                            out_t[:, mc, :] if out_t is not kt
                            else kt[:, mc, sb * SB:(sb + 1) * SB],
                            ps[:], bias_t[:, mc:mc + 1],
                        )
                return run

            out = []
            for mc in range(NMC):
                box_q, box_k = [None], [None]
                out.append((4 * SB * U, None, qk_half(wq_t, bqt, qt, mc, 0, box_q)))
                out.append((4 * SB * U, None, qk_half(wq_t, bqt, qt, mc, 1, box_q)))
                out.append((4 * SB * U, None, qk_half(wk_t, bkt, kt, mc, 0, box_k)))
                out.append((4 * SB * U, None, qk_half(wk_t, bkt, kt, mc, 1, box_k)))
            return out

        def v_units(sb):
            """V projection for s-block sb: 8 (est, closure) units."""
            xt = xt_t[sb]

            def v_half(sc, half, ps_box):
                gsc = sb * (SB // P) + sc

                def run():
                    if half == 0:
                        ps_box[0] = pps.tile([P, SB], f32, name="ps")
                    ps = ps_box[0]
                    for dc in range(4 * half, 4 * half + 4):
                        nc.tensor.matmul(
                            ps[:],
                            (xt[:, dc, sc * P:(sc + 1) * P]),
                            (wv_t[:, dc, :]),
                            start=(dc == 0),
                            stop=(dc == NDC - 1),
                        )
                    if half == 1:
                        nc.vector.tensor_add(
                            vaug[:, gsc, :, 0:DK],
                            ps.rearrange("p (h d) -> p h d", h=HPC),
                            vb_sb[:],
                        )
                        nc.gpsimd.memset(vaug[:, gsc, :, DK:DK + 1], 1.0)
                return run

            out = []
            for sc in range(SB // P):
                box_v = [None]
                # deadline: B(hp0, c=4*sb+sc) consumes vaug chunk 4*sb+sc
                dl = 4 * sb + sc
                out.append((4 * SB * U, dl, v_half(sc, 0, box_v)))
                out.append((4 * SB * U, dl, v_half(sc, 1, box_v)))
            return out

        # yb copies run on DVE during the attention stream but alternate
        # DVE/ACT in the drain/tail region (both engines are otherwise idle
        # there; keeping them balanced avoids an in-order backlog on either
        # gating the tail's PSUM release).
        yb_eng = ["dve", 0]

        def yb_copy(yb, ps, dc):
            use_act = yb_eng[0] == "act" or (
                yb_eng[0] == "alt" and yb_eng[1] % 2 == 0
            )
            yb_eng[1] += 1
            if use_act:
                nc.scalar.activation(
                    yb, ps, AF.Identity, bias=bot[:, dc:dc + 1]
                )
            else:
                nc.vector.tensor_scalar_add(yb, ps, bot[:, dc:dc + 1])

        def wo_units(sb, ot):
            """Output projection for s-block sb: 8 (est, closure) units."""
            out = []

            def one(dc):
                def run():
                    ps = pps.tile([P, SB], f32, name="ps")
                    for hc in range(NMC):
                        nc.tensor.matmul(
                            ps[:],
                            (wo_bf[:, hc, dc * P:(dc + 1) * P]),
                            (ot[:, hc, :]),
                            start=(hc == 0),
                            stop=(hc == NMC - 1),
                        )
                    yb = ybp.tile([P, SB], bf16, name="yb")
                    yb_copy(yb[:], ps[:], dc)
                    nc.sync.dma_start(
                        YT[dc * P:(dc + 1) * P, sb * SB:(sb + 1) * SB], yb[:]
                    )
                return run

            for dc in range(NDC):
                out.append((4 * SB * U, None, one(dc)))
            return out

        # ---------- main phases --------------------------------------------
        deferred_wo = []          # (sb, ot) pairs whose WO is deferred to p3
        ACT_C = 0.8333            # ns per ACT element

        # Q/K proj(0) runs standalone (nothing else for the PE yet).
        qt_cur = qtp.tile([P, NMC, SB], bf16, name="qt")
        qk0 = qk_units(0, qt_cur)
        # mc0 halves interleaved Q-h0, K-h0, Q-h1, K-h1: the h0 halves only
        # need xt dc0-3, so the PE isn't stuck behind the later xt dc4-7 DMA
        for j in (0, 2, 1, 3):
            qk0[j][2]()
        for _, _, u in qk0[4:]:
            u()

        for sb in range(NSB):
            qsb = sb
            # stream XT for sb+2 (xt pool bufs=2; sb,sb+1 already resident)
            if sb + 2 < NSB:
                xt_t[sb + 2] = xtp.tile([P, NDC, SB], bf16, name="xtblk")
                nc.sync.dma_start(xt_t[sb + 2][:], XT[:, :, (sb + 2) * SB:(sb + 3) * SB])

            # filler inventory for this phase: this block's V projection
            # (deadline-paced, just in time for the diagonal AVs), the next
            # block's Q/K projection, and in the last phase all deferred WO.
            nchunks_ = 4 * sb + 4
            n_slots_ = NMC * nchunks_
            fillers = deque()
            fillers.extend(v_units(sb))
            qt_next = None
            spread = []
            if sb + 1 < NSB:
                qt_next = qtp.tile([P, NMC, SB], bf16, name="qt")
                spread.extend(qk_units(sb + 1, qt_next))
            if sb == NSB - 1:
                for dsb, dot in deferred_wo:
                    spread.extend(wo_units(dsb, dot))
            # give budget-only units evenly-spread deadlines so none pile up
            # at the phase boundary; merge with the V deadlines sorted.
            nsp = len(spread)
            spread = [
                (est, min(n_slots_ - 2, (j + 1) * n_slots_ // (nsp + 1)), u)
                for j, (est, _, u) in enumerate(spread)
            ]
            fillers = deque(sorted(
                list(fillers) + spread, key=lambda t: (t[1], 0)
            ))

            qt = qt_cur
            osb = osbp.tile([P, 4, HPC, DK], bf16, name="osb")
            ot = otp.tile([P, NMC, SB], bf16, name="ot")

            nchunks = 4 * qsb + 4
            # deficit-paced filling: per chunk, ACT exp cost minus the PE
            # work of the chunk itself; scaled so the filler supply lasts
            # exactly to the end of the phase.
            def chunk_deficit(c):
                i = c - 4 * qsb
                s0 = 0 if i < 1 else P * i
                n_av = 2 * (4 - max(i, 0))
                act = 2 * (SB - s0) * ACT_C + 185.0
                pe = (2 * (SB - s0) + n_av * 65) * U
                return max(act - pe, 0.0)

            tot_deficit = sum(chunk_deficit(c) for c in range(nchunks)) * NMC
            tot_fill = sum(est for est, _, _ in fillers)
            dscale = min(1.0, tot_fill / max(tot_deficit, 1.0))
            budget = [0.0]

            def pop_filler():
                est, _, u = fillers.popleft()
                u()
                budget[0] -= est

            def fill(d, c_slot):
                budget[0] += d * dscale
                while fillers and fillers[0][1] is not None and fillers[0][1] <= c_slot:
                    pop_filler()
                while fillers and budget[0] >= fillers[0][0] * 0.5:
                    pop_filler()

            def emit_a(hp, c):
                i = c - 4 * qsb
                s0 = 0 if i < 1 else P * i
                sp = psp.tile([P, 2, SB], f32, tag="sp")
                for e in range(2):
                    off = e * DK
                    nc.tensor.matmul(
                        sp[:, e, s0:],
                        (kt[off:off + DK, hp, c * P:(c + 1) * P]),
                        (qt[off:off + DK, hp, s0:]),
                        start=True,
                        stop=True,
                    )
                at_g = attnp.tile([P, 2, SB], bf16)
                nc.scalar.activation(
                    at_g[:, :, s0:], sp[:, :, s0:], AF.Exp, scale=0.125
                )
                if i >= 0:
                    d0 = P * i
                    for e in range(2):
                        nc.vector.tensor_mul(
                            at_g[:, e, d0:d0 + P],
                            at_g[:, e, d0:d0 + P],
                            masks_b[:],
                        )
                return at_g

            o_map = {}

            def make_b(hp, c, at_g):
                i = c - 4 * qsb

                def run():
                    if c == 0:
                        o_map[hp] = [
                            pav.tile([P, 4, DK + 1], f32, name="oacc")
                            for _ in range(2)
                        ]
                    o_e = o_map[hp]
                    # One PSUM accumulation group per (hp, e) bank: start=1
                    # lazily zeroes the whole 2KB zero region, so only the
                    # very first matmul starts and only the last one stops.
                    # (Interleaved per-jj groups in one bank corrupt on HW.)
                    for e in range(2):
                        for jj in range(max(i, 0), 4):
                            nc.tensor.matmul(
                                o_e[e][:, jj, :],
                                (at_g[:, e, jj * P:(jj + 1) * P]),
                                (vaug[:, c, 2 * hp + e, :]),
                                start=(c == 0 and jj == 0),
                                stop=(c == 4 * qsb + 3),
                            )
                return run

            def epi_norm(hp):
                # reciprocal + normalize for one head pair (DVE)
                o_e = o_map.pop(hp)
                rec = recp.tile([P, 2, 4], f32)
                for e in range(2):
                    nc.vector.reciprocal(rec[:, e, :], o_e[e][:, :, DK])
                for e in range(2):
                    for jj in range(4):
                        nc.vector.tensor_scalar_mul(
                            osb[:, jj, 2 * hp + e, :],
                            o_e[e][:, jj, 0:DK],
                            rec[:, e, jj:jj + 1],
                        )

            def epi_transpose(hp):
                # O -> O^T. All but the very last head pair go through the
                # XBAR transpose DMA on the (mostly idle) SP queue - zero PE
                # rows and nothing added to the DVE queue that gates the AV
                # accumulator recycling. The final pair of the final phase
                # feeds the tail WO immediately, so it keeps the low-latency
                # PE-transpose + DVE-copy path.
                if sb == NSB - 1 and hp == NMC - 1:
                    tp = pps.tile([P, SB], f32, name="ps")
                    tpb = tp[:].bitcast(bf16)
                    for jj in range(4):
                        nc.tensor.matmul(
                            tpb[:, jj * P:(jj + 1) * P],
                            osb[:, jj, 2 * hp:2 * hp + 2, :],
                            ident_b[:],
                            is_transpose=True,
                            start=(jj == 0),
                            stop=(jj == 3),
                        )
                    nc.vector.tensor_scalar_add(ot[:, hp, :], tpb[:, 0:SB], 0.0)
                else:
                    for jj in range(4):
                        nc.sync.dma_start_transpose(
                            ot[:, hp, jj * P:(jj + 1) * P],
                            osb[:, jj, 2 * hp:2 * hp + 2, :],
                        )

            # cross-hp pipelined chunk stream: B(k) is emitted after A(k+1)
            # (after A(k+2) for each hp's first chunk, giving the PSUM
            # accumulator pool an extra slot of slack to absorb the previous
            # hp's DVE normalization latency), and each hp's epilogue right
            # after its last B, which already overlaps the next hp's scores.
            pend_b = deque()   # (hp, c, closure, emit_idx)
            pending_tp = deque()
            idx = 0

            def run_due_b(cur_idx):
                while pend_b:
                    bhp, bc, bb, ei = pend_b[0]
                    need = 1
                    if cur_idx - ei < need:
                        break
                    pend_b.popleft()
                    bb()
                    if bc == nchunks - 1:
                        epi_norm(bhp)
                        pending_tp.append((cur_idx + 4, bhp))

            for hp in range(NMC):
                for c in range(nchunks):
                    at_g = emit_a(hp, c)
                    fill(chunk_deficit(c), idx)
                    while pending_tp and pending_tp[0][0] <= idx:
                        epi_transpose(pending_tp.popleft()[1])
                    run_due_b(idx)
                    pend_b.append((hp, c, make_b(hp, c, at_g), idx))
                    idx += 1
            fill(500.0, NMC * nchunks)
            while pend_b:
                bhp, bc, bb, ei = pend_b.popleft()
                bb()
                if bc == nchunks - 1 and bhp < NMC - 1:
                    epi_norm(bhp)
                    pending_tp.append((idx, bhp))
            epi_norm(NMC - 1)
            while pending_tp:
                epi_transpose(pending_tp.popleft()[1])
            yb_last = yb_eng[0]
            if sb == NSB - 1:
                yb_eng[0] = "alt"
            for _ in range(2):
                if fillers:
                    pop_filler()
            epi_transpose(NMC - 1)

            # drain leftover fillers
            while fillers:
                pop_filler()
            if sb != NSB - 1:
                yb_eng[0] = yb_last

            deferred_wo.append((sb, ot))
            qt_cur = qt_next

        # tail: WO for the last s-block, pipelined 4-deep through the
        # (now idle) scores PSUM pool so pool rotation never stalls the PE.
        # yb copies on ACT (idle); the last dc split in halves so the final
        # YT DMA starts as early as possible.
        sb3, ot3 = deferred_wo[-1]
        ps_box = [None]
        for dc in range(NDC):
            if dc % 2 == 0:
                ps_box[0] = psp.tile([P, 2, SB], f32, tag="sp", name="sp")
            ps = ps_box[0][:, dc % 2, :]
            for hc in range(NMC):
                nc.tensor.matmul(
                    ps,
                    (wo_bf[:, hc, dc * P:(dc + 1) * P]),
                    (ot3[:, hc, :]),
                    start=(hc == 0),
                    stop=(hc == NMC - 1),
                )
            nparts = 1
            w = SB // nparts
            for h in range(nparts):
                hs = slice(h * w, (h + 1) * w)
                # separate tile per piece: same-tile writers on different
                # engines serialize via semaphores otherwise
                yb = ybp.tile([P, w], bf16, name="yb")
                yb_copy(yb[:], ps[:, hs], dc)
                nc.sync.dma_start(
                    YT[dc * P:(dc + 1) * P,
                       sb3 * SB + h * w:sb3 * SB + (h + 1) * w],
                    yb[:],
                )
        # earlier blocks' WO ran as fillers in phase 3
    nc.finalize()
    return nc


def _masks():
    p = np.arange(P)[:, None]
    j = np.arange(P)[None, :]
    return (p <= j).astype(np.float32)


def _in_maps(X, Wq, bq, Wk, bk, Wv, bv, Wo, bo):
    import ml_dtypes
    bf = ml_dtypes.bfloat16
    masks = _masks().astype(bf)                       # [P, P]
    ident = np.eye(P, dtype=np.float32).astype(bf)    # [P, P]
    zeros_bo = np.zeros_like(bo)

    def pre_qk(w):   # [D, MD] -> [P, NMC, NDC, P] (mc-major, per-part contig)
        return np.ascontiguousarray(
            w.reshape(NDC, P, NMC, P).transpose(1, 2, 0, 3).astype(bf))

    def pre_dm(w):   # [D, MD] -> [P, NDC, MD]
        return np.ascontiguousarray(
            w.reshape(NDC, P, MD).transpose(1, 0, 2).astype(bf))

    maps = []
    for core in range(8):
        b, hg = core // 2, core % 2
        sl = slice(hg * MD, (hg + 1) * MD)
        bo_c = bo if hg == 0 else zeros_bo
        cf32 = np.concatenate([
            bq[sl].reshape(NMC, P).T,                 # [P, NMC]
            bk[sl].reshape(NMC, P).T,                 # [P, NMC]
            bo_c.reshape(NDC, P).T,                   # [P, NDC]
        ], axis=1).astype(np.float32)
        cbf = np.concatenate([
            masks, ident,
            np.broadcast_to(bv[sl].astype(bf), (P, MD)),
        ], axis=1).astype(bf)
        maps.append({
            "XT": np.ascontiguousarray(
                X[b].T.reshape(NDC, P, S).transpose(1, 0, 2).astype(bf)),
            "WQ": pre_qk(Wq[:, sl]),
            "WK": pre_qk(Wk[:, sl]),
            "WV": pre_dm(Wv[:, sl]),
            "WO": np.ascontiguousarray(
                Wo[sl, :].reshape(NMC, P, D).transpose(1, 0, 2).astype(bf)),
            "CF32": np.ascontiguousarray(cf32),
            "CBF": np.ascontiguousarray(cbf),
        })
    return maps


_LAST_RESULTS = None


def kernel(X, Wq, bq, Wk, bk, Wv, bv, Wo, bo):
    global _LAST_RESULTS
    _ensure_path()
    from concourse import bass_utils

    args = [np.ascontiguousarray(np.asarray(a, dtype=np.float32))
            for a in (X, Wq, bq, Wk, bk, Wv, bv, Wo, bo)]
    if "nc" not in _CACHE:
        _CACHE["nc"] = _build()
    nc = _CACHE["nc"]
    res = bass_utils.run_bass_kernel_spmd(nc, _in_maps(*args), core_ids=list(range(8)))
    _LAST_RESULTS = res
    out = np.empty((B, S, D), dtype=np.float32)
    for b in range(B):
        out[b] = (res.results[2 * b]["YT"] + res.results[2 * b + 1]["YT"]).T
    return out


# revision 52
# speedup vs baseline: 1.0001x; 1.0001x over previous
"""Trainium2 Bass kernel for 16-head causal MHA (B=4, S=2048, D=1024).

Sharding: 8 cores = 4 batches x 2 head-groups (8 heads each).
Per core (batch b, head-group hg):
  inputs:  XT = X[b].T [1024,2048] bf16, WQ/WK/WV column shards [1024,512]
           bf16, WO row shard [512,1024] bf16, bias shards, causal mask tile.
  output:  YT = (O_hg @ WO_hg + bo*[hg==0]).T  [1024, 2048]  (partial)
Host combine: Y[b] = (YT[2b] + YT[2b+1]).T

On-core dataflow (all-bf16 matmul pipeline, fp32 PSUM):
  Q^T,K^T [512,2048] and V [2048,512] via bf16 matmuls.
  scores^T[sk,sq] = K_h @ Q_h^T (bf16, exact causal diagonal trim); exp on
  ACT (scale=1/8 folded) -> A^T in bf16; causal mask multiplied on the
  diagonal 128-block (DVE).
  AV in the M=128 form: O[sq,dv+1] += A^T-chunk(stationary) @ [V|1](moving)
  in bf16 (N=65); col 64 gives the softmax denominators for free.
  Normalize on DVE (per-partition reciprocal scale), O -> O^T via PE
  transposes (identity; 4 blocks share one lazily-zeroed PSUM bank),
  Y^T = WO^T @ O^T in bf16.
  Projection / output-projection matmuls are interleaved into the attention
  chunk stream as fillers so the PE never stalls on the ACT exp chain.
"""

import sys
from collections import deque

import numpy as np

_REPO = "/opt/trn_rl_repo"

B, S, D = 4, 2048, 1024
H, DK = 16, 64
HPC = 8            # heads per core
MD = HPC * DK      # 512: per-core head width
P = 128
SB = 512           # s-block
NSB = S // SB      # 4
NDC = D // P       # 8
NMC = MD // P      # 4
NSC = S // P       # 16

_CACHE = {}


def _ensure_path():
    try:
        import concourse  # noqa: F401
    except ImportError:
        if _REPO not in sys.path:
            sys.path.insert(0, _REPO)


def _build():
    _ensure_path()
    from contextlib import ExitStack

    import concourse.bass as bass  # noqa: F401
    import concourse.mybir as mybir
    import concourse.tile as tile
    from concourse import bacc

    dt = mybir.dt
    f32 = dt.float32
    bf16 = dt.bfloat16
    AF = mybir.ActivationFunctionType

    nc = bacc.Bacc(None, target_bir_lowering=False)
    # All inputs host-pre-laid into per-partition-contiguous layouts so every
    # load is one fat DMA (descriptor rows >= 512B avoid the 2x RMW penalty;
    # each DMA costs ~650ns sequencer + ~900ns semaphore regardless of size).
    XT = nc.dram_tensor("XT", [P, NDC, S], bf16, kind="ExternalInput")
    WQ = nc.dram_tensor("WQ", [P, NMC, NDC, P], bf16, kind="ExternalInput")
    WK = nc.dram_tensor("WK", [P, NMC, NDC, P], bf16, kind="ExternalInput")
    WV = nc.dram_tensor("WV", [P, NDC, MD], bf16, kind="ExternalInput")
    WO = nc.dram_tensor("WO", [P, NMC, D], bf16, kind="ExternalInput")
    # packed consts: CF32 = [bq | bk | bo] per-partition cols; CBF =
    # [masks | ident | vb-broadcast]
    CF32 = nc.dram_tensor("CF32", [P, 2 * NMC + NDC], f32, kind="ExternalInput")
    CBF = nc.dram_tensor("CBF", [P, 2 * P + MD], bf16, kind="ExternalInput")
    YT = nc.dram_tensor("YT", [D, S], bf16, kind="ExternalOutput")

    with ExitStack() as ctx:
        ctx.enter_context(nc.allow_low_precision(reason="bf16 pipeline"))
        tc = ctx.enter_context(tile.TileContext(nc))
        consts = ctx.enter_context(tc.tile_pool(name="consts", bufs=1))
        qkv = ctx.enter_context(tc.tile_pool(name="qkv", bufs=1))
        wst = ctx.enter_context(tc.tile_pool(name="wst", bufs=1))
        xtp = ctx.enter_context(tc.tile_pool(name="xt", bufs=2))
        qtp = ctx.enter_context(tc.tile_pool(name="qt", bufs=2))
        attnp = ctx.enter_context(tc.tile_pool(name="attn", bufs=3))
        osbp = ctx.enter_context(tc.tile_pool(name="osb", bufs=2))
        otp = ctx.enter_context(tc.tile_pool(name="ot", bufs=4))
        recp = ctx.enter_context(tc.tile_pool(name="rec", bufs=4))
        ybp = ctx.enter_context(tc.tile_pool(name="yb", bufs=8))
        pps = ctx.enter_context(tc.tile_pool(name="pps", bufs=2, space="PSUM"))
        psp = ctx.enter_context(tc.tile_pool(name="psp", bufs=2, space="PSUM"))
        pav = ctx.enter_context(tc.tile_pool(name="pav", bufs=2, space="PSUM"))

        # Dummy first ACT op: walrus attaches the ACT table-load pseudo to the
        # first activation; keep its sync-wait list minimal.
        dummy = consts.tile([1, 16], f32)
        nc.vector.memset(dummy[:], 0.0)
        nc.scalar.activation(dummy[:], dummy[:], AF.Exp)
        nc.scalar.activation(dummy[:], dummy[:], AF.Identity)

        kt = qkv.tile([P, NMC, S], bf16)              # K^T  (m-chunk, sk)
        vaug = qkv.tile([P, NSC, HPC, DK + 1], bf16)  # V per s-chunk + ones col
        wo_bf = qkv.tile([P, NMC, D], bf16)

        # ---------- setup DMAs, ordered so the PE starts early --------------
        # Two HWDGE queues: ACT (idle until the first exp at ~10us) carries
        # the first W strips + consts; SP carries the xt stream and all
        # later bulk loads so the exp stream never queues behind a DMA.
        xt_t = [None] * NSB
        xt_t[0] = xtp.tile([P, NDC, SB], bf16, name="xtblk")
        wq_t = wst.tile([P, NMC, NDC, P], bf16, tag="wq")
        wk_t = wst.tile([P, NMC, NDC, P], bf16, tag="wk")
        wv_t = wst.tile([P, NDC, MD], bf16, tag="wv")

        nc.sync.dma_start(xt_t[0][:, 0, :], XT[:, 0, 0:SB])
        nc.scalar.dma_start(wq_t[:, 0, 0:4, :], WQ[:, 0, 0:4, :])
        nc.sync.dma_start(xt_t[0][:, 1, :], XT[:, 1, 0:SB])
        nc.scalar.dma_start(wq_t[:, 0, 4:NDC, :], WQ[:, 0, 4:NDC, :])
        nc.sync.dma_start(xt_t[0][:, 2:4, :], XT[:, 2:4, 0:SB])
        nc.scalar.dma_start(wk_t[:, 0, :, :], WK[:, 0, :, :])
        nc.sync.dma_start(xt_t[0][:, 4:NDC, :], XT[:, 4:NDC, 0:SB])
        cf = consts.tile([P, 2 * NMC + NDC], f32)
        nc.scalar.dma_start(cf[:], CF32[:, :])
        bqt = cf[:, 0:NMC]
        bkt = cf[:, NMC:2 * NMC]
        bot = cf[:, 2 * NMC:]
        nc.scalar.dma_start(wq_t[:, 1, :, :], WQ[:, 1, :, :])
        nc.scalar.dma_start(wk_t[:, 1, :, :], WK[:, 1, :, :])
        cb = consts.tile([P, 2 * P + MD], bf16)
        nc.scalar.dma_start(cb[:], CBF[:, :])
        masks_b = cb[:, 0:P]
        ident_b = cb[:, P:2 * P]
        vb_sb = cb[:, 2 * P:].rearrange("p (h d) -> p h d", h=HPC)
        # remaining bulk loads on SP, in PE-consumption order
        nc.sync.dma_start(wq_t[:, 2, :, :], WQ[:, 2, :, :])
        nc.sync.dma_start(wk_t[:, 2, :, :], WK[:, 2, :, :])
        nc.sync.dma_start(wq_t[:, 3, :, :], WQ[:, 3, :, :])
        nc.sync.dma_start(wk_t[:, 3, :, :], WK[:, 3, :, :])
        nc.sync.dma_start(wv_t[:, 0:4, :], WV[:, 0:4, :])
        xt_t[1] = xtp.tile([P, NDC, SB], bf16, name="xtblk")
        nc.sync.dma_start(xt_t[1][:, 0:4, :], XT[:, 0:4, SB:2 * SB])
        nc.sync.dma_start(wv_t[:, 4:NDC, :], WV[:, 4:NDC, :])
        nc.sync.dma_start(xt_t[1][:, 4:NDC, :], XT[:, 4:NDC, SB:2 * SB])
        nc.sync.dma_start(wo_bf[:], WO[:, :, :])

        # ---------- emitters ------------------------------------------------
        # filler units: (estimated_pe_ns, deadline, closure)
        U = 0.41667  # ns per PE row at full clock

        def qk_units(sb, qt):
            """Q/K projection for s-block sb: 16 (est, closure) units."""
            xt = xt_t[sb]

            def qk_half(w_t, bias_t, out_t, mc, half, ps_box):
                def run():
                    if half == 0:
                        ps_box[0] = pps.tile([P, SB], f32, name="ps")
                    ps = ps_box[0]
                    for dc in range(4 * half, 4 * half + 4):
                        nc.tensor.matmul(
                            ps[:],
                            (w_t[:, mc, dc, :]),
                            (xt[:, dc, :]),
                            start=(dc == 0),
                            stop=(dc == NDC - 1),
                        )
                    if half == 1:
                        nc.vector.tensor_scalar_add(
                            out_t[:, mc, :] if out_t is not kt
                            else kt[:, mc, sb * SB:(sb + 1) * SB],
                            ps[:], bias_t[:, mc:mc + 1],
                        )
                return run

            out = []
            for mc in range(NMC):
                box_q, box_k = [None], [None]
                out.append((4 * SB * U, None, qk_half(wq_t, bqt, qt, mc, 0, box_q)))
                out.append((4 * SB * U, None, qk_half(wq_t, bqt, qt, mc, 1, box_q)))
                out.append((4 * SB * U, None, qk_half(wk_t, bkt, kt, mc, 0, box_k)))
                out.append((4 * SB * U, None, qk_half(wk_t, bkt, kt, mc, 1, box_k)))
            return out

        def v_units(sb):
            """V projection for s-block sb: 8 (est, closure) units."""
            xt = xt_t[sb]

            def v_half(sc, half, ps_box):
                gsc = sb * (SB // P) + sc

                def run():
                    if half == 0:
                        ps_box[0] = pps.tile([P, SB], f32, name="ps")
                    ps = ps_box[0]
                    for dc in range(4 * half, 4 * half + 4):
                        nc.tensor.matmul(
                            ps[:],
                            (xt[:, dc, sc * P:(sc + 1) * P]),
                            (wv_t[:, dc, :]),
                            start=(dc == 0),
                            stop=(dc == NDC - 1),
                        )
                    if half == 1:
                        nc.vector.tensor_add(
                            vaug[:, gsc, :, 0:DK],
                            ps.rearrange("p (h d) -> p h d", h=HPC),
                            vb_sb[:],
                        )
                        nc.gpsimd.memset(vaug[:, gsc, :, DK:DK + 1], 1.0)
                return run

            out = []
            for sc in range(SB // P):
                box_v = [None]
                # deadline: B(hp0, c=4*sb+sc) consumes vaug chunk 4*sb+sc
                dl = 4 * sb + sc
                out.append((4 * SB * U, dl, v_half(sc, 0, box_v)))
                out.append((4 * SB * U, dl, v_half(sc, 1, box_v)))
            return out

        # yb copies run on DVE during the attention stream but alternate
        # DVE/ACT in the drain/tail region (both engines are otherwise idle
        # there; keeping them balanced avoids an in-order backlog on either
        # gating the tail's PSUM release).
        yb_eng = ["dve", 0]

        def yb_copy(yb, ps, dc):
            use_act = yb_eng[0] == "act" or (
                yb_eng[0] == "alt" and yb_eng[1] % 2 == 0
            )
            yb_eng[1] += 1
            if use_act:
                nc.scalar.activation(
                    yb, ps, AF.Identity, bias=bot[:, dc:dc + 1]
                )
            else:
                nc.vector.tensor_scalar_add(yb, ps, bot[:, dc:dc + 1])

        def wo_units(sb, ot):
            """Output projection for s-block sb: 8 (est, closure) units."""
            out = []

            def one(dc):
                def run():
                    ps = pps.tile([P, SB], f32, name="ps")
                    for hc in range(NMC):
                        nc.tensor.matmul(
                            ps[:],
                            (wo_bf[:, hc, dc * P:(dc + 1) * P]),
                            (ot[:, hc, :]),
                            start=(hc == 0),
                            stop=(hc == NMC - 1),
                        )
                    yb = ybp.tile([P, SB], bf16, name="yb")
                    yb_copy(yb[:], ps[:], dc)
                    nc.sync.dma_start(
                        YT[dc * P:(dc + 1) * P, sb * SB:(sb + 1) * SB], yb[:]
                    )
                return run

            for dc in range(NDC):
                out.append((4 * SB * U, None, one(dc)))
            return out

        # ---------- main phases --------------------------------------------
        deferred_wo = []          # (sb, ot) pairs whose WO is deferred to p3
        ACT_C = 0.8333            # ns per ACT element

        # Q/K proj(0) runs standalone (nothing else for the PE yet).
        qt_cur = qtp.tile([P, NMC, SB], bf16, name="qt")
        for _, _, u in qk_units(0, qt_cur):
            u()

        for sb in range(NSB):
            qsb = sb
            # stream XT for sb+2 (xt pool bufs=2; sb,sb+1 already resident)
            if sb + 2 < NSB:
                xt_t[sb + 2] = xtp.tile([P, NDC, SB], bf16, name="xtblk")
                nc.sync.dma_start(xt_t[sb + 2][:], XT[:, :, (sb + 2) * SB:(sb + 3) * SB])

            # filler inventory for this phase: this block's V projection
            # (deadline-paced, just in time for the diagonal AVs), the next
            # block's Q/K projection, and in the last phase all deferred WO.
            nchunks_ = 4 * sb + 4
            n_slots_ = NMC * nchunks_
            fillers = deque()
            fillers.extend(v_units(sb))
            qt_next = None
            spread = []
            if sb + 1 < NSB:
                qt_next = qtp.tile([P, NMC, SB], bf16, name="qt")
                spread.extend(qk_units(sb + 1, qt_next))
            if sb == NSB - 1:
                for dsb, dot in deferred_wo:
                    spread.extend(wo_units(dsb, dot))
            # give budget-only units evenly-spread deadlines so none pile up
            # at the phase boundary; merge with the V deadlines sorted.
            nsp = len(spread)
            spread = [
                (est, min(n_slots_ - 2, (j + 1) * n_slots_ // (nsp + 1)), u)
                for j, (est, _, u) in enumerate(spread)
            ]
            fillers = deque(sorted(
                list(fillers) + spread, key=lambda t: (t[1], 0)
            ))

            qt = qt_cur
            osb = osbp.tile([P, 4, HPC, DK], bf16, name="osb")
            ot = otp.tile([P, NMC, SB], bf16, name="ot")

            nchunks = 4 * qsb + 4
            # deficit-paced filling: per chunk, ACT exp cost minus the PE
            # work of the chunk itself; scaled so the filler supply lasts
            # exactly to the end of the phase.
            def chunk_deficit(c):
                i = c - 4 * qsb
                s0 = 0 if i < 1 else P * i
                n_av = 2 * (4 - max(i, 0))
                act = 2 * (SB - s0) * ACT_C + 185.0
                pe = (2 * (SB - s0) + n_av * 65) * U
                return max(act - pe, 0.0)

            tot_deficit = sum(chunk_deficit(c) for c in range(nchunks)) * NMC
            tot_fill = sum(est for est, _, _ in fillers)
            dscale = min(1.0, tot_fill / max(tot_deficit, 1.0))
            budget = [0.0]

            def pop_filler():
                est, _, u = fillers.popleft()
                u()
                budget[0] -= est

            def fill(d, c_slot):
                budget[0] += d * dscale
                while fillers and fillers[0][1] is not None and fillers[0][1] <= c_slot:
                    pop_filler()
                while fillers and budget[0] >= fillers[0][0] * 0.5:
                    pop_filler()

            def emit_a(hp, c):
                i = c - 4 * qsb
                s0 = 0 if i < 1 else P * i
                sp = psp.tile([P, 2, SB], f32, tag="sp")
                for e in range(2):
                    off = e * DK
                    nc.tensor.matmul(
                        sp[:, e, s0:],
                        (kt[off:off + DK, hp, c * P:(c + 1) * P]),
                        (qt[off:off + DK, hp, s0:]),
                        start=True,
                        stop=True,
                    )
                at_g = attnp.tile([P, 2, SB], bf16)
                nc.scalar.activation(
                    at_g[:, :, s0:], sp[:, :, s0:], AF.Exp, scale=0.125
                )
                if i >= 0:
                    d0 = P * i
                    for e in range(2):
                        nc.vector.tensor_mul(
                            at_g[:, e, d0:d0 + P],
                            at_g[:, e, d0:d0 + P],
                            masks_b[:],
                        )
                return at_g

            o_map = {}

            def make_b(hp, c, at_g):
                i = c - 4 * qsb

                def run():
                    if c == 0:
                        o_map[hp] = [
                            pav.tile([P, 4, DK + 1], f32, name="oacc")
                            for _ in range(2)
                        ]
                    o_e = o_map[hp]
                    # One PSUM accumulation group per (hp, e) bank: start=1
                    # lazily zeroes the whole 2KB zero region, so only the
                    # very first matmul starts and only the last one stops.
                    # (Interleaved per-jj groups in one bank corrupt on HW.)
                    for e in range(2):
                        for jj in range(max(i, 0), 4):
                            nc.tensor.matmul(
                                o_e[e][:, jj, :],
                                (at_g[:, e, jj * P:(jj + 1) * P]),
                                (vaug[:, c, 2 * hp + e, :]),
                                start=(c == 0 and jj == 0),
                                stop=(c == 4 * qsb + 3),
                            )
                return run

            def epi_norm(hp):
                # reciprocal + normalize for one head pair (DVE)
                o_e = o_map.pop(hp)
                rec = recp.tile([P, 2, 4], f32)
                for e in range(2):
                    nc.vector.reciprocal(rec[:, e, :], o_e[e][:, :, DK])
                for e in range(2):
                    for jj in range(4):
                        nc.vector.tensor_scalar_mul(
                            osb[:, jj, 2 * hp + e, :],
                            o_e[e][:, jj, 0:DK],
                            rec[:, e, jj:jj + 1],
                        )

            def epi_transpose(hp):
                # O -> O^T. All but the very last head pair go through the
                # XBAR transpose DMA on the (mostly idle) SP queue - zero PE
                # rows and nothing added to the DVE queue that gates the AV
                # accumulator recycling. The final pair of the final phase
                # feeds the tail WO immediately, so it keeps the low-latency
                # PE-transpose + DVE-copy path.
                if sb == NSB - 1 and hp == NMC - 1:
                    tp = pps.tile([P, SB], f32, name="ps")
                    tpb = tp[:].bitcast(bf16)
                    for jj in range(4):
                        nc.tensor.matmul(
                            tpb[:, jj * P:(jj + 1) * P],
                            osb[:, jj, 2 * hp:2 * hp + 2, :],
                            ident_b[:],
                            is_transpose=True,
                            start=(jj == 0),
                            stop=(jj == 3),
                        )
                    nc.vector.tensor_scalar_add(ot[:, hp, :], tpb[:, 0:SB], 0.0)
                else:
                    for jj in range(4):
                        nc.sync.dma_start_transpose(
                            ot[:, hp, jj * P:(jj + 1) * P],
                            osb[:, jj, 2 * hp:2 * hp + 2, :],
                        )

            # cross-hp pipelined chunk stream: B(k) is emitted after A(k+1)
            # (after A(k+2) for each hp's first chunk, giving the PSUM
            # accumulator pool an extra slot of slack to absorb the previous
            # hp's DVE normalization latency), and each hp's epilogue right
            # after its last B, which already overlaps the next hp's scores.
            pend_b = deque()   # (hp, c, closure, emit_idx)
            pending_tp = deque()
            idx = 0

            def run_due_b(cur_idx):
                while pend_b:
                    bhp, bc, bb, ei = pend_b[0]
                    need = 1
                    if cur_idx - ei < need:
                        break
                    pend_b.popleft()
                    bb()
                    if bc == nchunks - 1:
                        epi_norm(bhp)
                        pending_tp.append((cur_idx + 4, bhp))

            for hp in range(NMC):
                for c in range(nchunks):
                    at_g = emit_a(hp, c)
                    fill(chunk_deficit(c), idx)
                    while pending_tp and pending_tp[0][0] <= idx:
                        epi_transpose(pending_tp.popleft()[1])
                    run_due_b(idx)
                    pend_b.append((hp, c, make_b(hp, c, at_g), idx))
                    idx += 1
            fill(500.0, NMC * nchunks)
            while pend_b:
                bhp, bc, bb, ei = pend_b.popleft()
                bb()
                if bc == nchunks - 1 and bhp < NMC - 1:
                    epi_norm(bhp)
                    pending_tp.append((idx, bhp))
            epi_norm(NMC - 1)
            while pending_tp:
                epi_transpose(pending_tp.popleft()[1])
            yb_last = yb_eng[0]
            if sb == NSB - 1:
                yb_eng[0] = "alt"
            for _ in range(2):
                if fillers:
                    pop_filler()
            epi_transpose(NMC - 1)

            # drain leftover fillers
            while fillers:
                pop_filler()
            if sb != NSB - 1:
                yb_eng[0] = yb_last

            deferred_wo.append((sb, ot))
            qt_cur = qt_next

        # tail: WO for the last s-block, pipelined 4-deep through the
        # (now idle) scores PSUM pool so pool rotation never stalls the PE.
        # yb copies on ACT (idle); the last dc split in halves so the final
        # YT DMA starts as early as possible.
        sb3, ot3 = deferred_wo[-1]
        ps_box = [None]
        for dc in range(NDC):
            if dc % 2 == 0:
                ps_box[0] = psp.tile([P, 2, SB], f32, tag="sp", name="sp")
            ps = ps_box[0][:, dc % 2, :]
            for hc in range(NMC):
                nc.tensor.matmul(
                    ps,
                    (wo_bf[:, hc, dc * P:(dc + 1) * P]),
                    (ot3[:, hc, :]),
                    start=(hc == 0),
                    stop=(hc == NMC - 1),
                )
            nparts = 1
            w = SB // nparts
            for h in range(nparts):
                hs = slice(h * w, (h + 1) * w)
                # separate tile per piece: same-tile writers on different
                # engines serialize via semaphores otherwise
                yb = ybp.tile([P, w], bf16, name="yb")
                yb_copy(yb[:], ps[:, hs], dc)
                nc.sync.dma_start(
                    YT[dc * P:(dc + 1) * P,
                       sb3 * SB + h * w:sb3 * SB + (h + 1) * w],
                    yb[:],
                )
        # earlier blocks' WO ran as fillers in phase 3
    nc.finalize()
    return nc


def _masks():
    p = np.arange(P)[:, None]
    j = np.arange(P)[None, :]
    return (p <= j).astype(np.float32)


def _in_maps(X, Wq, bq, Wk, bk, Wv, bv, Wo, bo):
    import ml_dtypes
    bf = ml_dtypes.bfloat16
    masks = _masks().astype(bf)                       # [P, P]
    ident = np.eye(P, dtype=np.float32).astype(bf)    # [P, P]
    zeros_bo = np.zeros_like(bo)

    def pre_qk(w):   # [D, MD] -> [P, NMC, NDC, P] (mc-major, per-part contig)
        return np.ascontiguousarray(
            w.reshape(NDC, P, NMC, P).transpose(1, 2, 0, 3).astype(bf))

    def pre_dm(w):   # [D, MD] -> [P, NDC, MD]
        return np.ascontiguousarray(
            w.reshape(NDC, P, MD).transpose(1, 0, 2).astype(bf))

    maps = []
    for core in range(8):
        b, hg = core // 2, core % 2
        sl = slice(hg * MD, (hg + 1) * MD)
        bo_c = bo if hg == 0 else zeros_bo
        cf32 = np.concatenate([
            bq[sl].reshape(NMC, P).T,                 # [P, NMC]
            bk[sl].reshape(NMC, P).T,                 # [P, NMC]
            bo_c.reshape(NDC, P).T,                   # [P, NDC]
        ], axis=1).astype(np.float32)
        cbf = np.concatenate([
            masks, ident,
            np.broadcast_to(bv[sl].astype(bf), (P, MD)),
        ], axis=1).astype(bf)
        maps.append({
            "XT": np.ascontiguousarray(
                X[b].T.reshape(NDC, P, S).transpose(1, 0, 2).astype(bf)),
            "WQ": pre_qk(Wq[:, sl]),
            "WK": pre_qk(Wk[:, sl]),
            "WV": pre_dm(Wv[:, sl]),
            "WO": np.ascontiguousarray(
                Wo[sl, :].reshape(NMC, P, D).transpose(1, 0, 2).astype(bf)),
            "CF32": np.ascontiguousarray(cf32),
            "CBF": np.ascontiguousarray(cbf),
        })
    return maps


_LAST_RESULTS = None


def kernel(X, Wq, bq, Wk, bk, Wv, bv, Wo, bo):
    global _LAST_RESULTS
    _ensure_path()
    from concourse import bass_utils

    args = [np.ascontiguousarray(np.asarray(a, dtype=np.float32))
            for a in (X, Wq, bq, Wk, bk, Wv, bv, Wo, bo)]
    if "nc" not in _CACHE:
        _CACHE["nc"] = _build()
    nc = _CACHE["nc"]
    res = bass_utils.run_bass_kernel_spmd(nc, _in_maps(*args), core_ids=list(range(8)))
    _LAST_RESULTS = res
    out = np.empty((B, S, D), dtype=np.float32)
    for b in range(B):
        out[b] = (res.results[2 * b]["YT"] + res.results[2 * b + 1]["YT"]).T
    return out
